# revision 16
# baseline (speedup 1.0000x reference)
"""GATv2Conv (DGL-style, H=4 heads, D=32) on 8 Trainium2 NeuronCores.

Self-contained: takes full inputs, shards internally, returns full output.

Strategy
--------
Host (numpy, index preprocessing only):
  * append self-loop edges, group edges by destination node
  * sort nodes by in-degree (desc), tile into 128-node blocks
  * deal blocks snake-wise across the 8 cores (edge-count balance <1%)
  * per block: a [128 nodes x L] grid of source indices (L = max degree in
    the round, shared across cores so all cores run one SPMD program)

Device (per core, one SPMD program):
  * phase A: fs = x @ W_src + b_src for ALL nodes -> DRAM table
             fd = x @ W_dst + b_dst for this core's nodes -> DRAM shard
  * phase B per 128-node block (grid [128 nodes x L edge slots]):
      fs_g = gather(fs, src)      (one [P,1]-offset SWDGE indirect DMA/slot)
      t    = fs_g + fd[node]      (DVE, broadcast over slots)
      u    = LeakyReLU(t)         (DVE scalar_tensor_tensor max(0.2t, t))
      v    = u * attn             (DVE)
      scr  = sum_d v              (DVE reduce)
      es   = exp(scr) * mask      (ACT + DVE)
      den  = sum_l es ; rden = 1/den                       (DVE)
      w    = es * fs_g            (DVE)
      agg  = sum_l w              (DVE reduce)
      out  = relu(agg * rden)     (DVE smalls)
  No segment max: scores are O(+-6) for this data regime, exp() is fp32-safe
  and softmax is shift-invariant, so results match the reference to fp32
  rounding.
"""

import os
from contextlib import ExitStack

import numpy as np

P = 128
H = 4
D = 32
HD = H * D  # 128
FIN = 128
CH = 512  # phase-A chunk of node rows
BUCKET_W = 32768  # dma_gather int16 index window
CAP = 36  # max grid columns processed per chunk (SBUF bound)
GCOLS = 8  # max columns (x128 descriptors) per dma_gather instruction


# --------------------------------------------------------------------------
# host-side graph plan (pure index preprocessing)
# --------------------------------------------------------------------------
def build_plan(src, dst, n_nodes, n_cores):
    s_all = np.concatenate([src.astype(np.int64), np.arange(n_nodes, dtype=np.int64)])
    d_all = np.concatenate([dst.astype(np.int64), np.arange(n_nodes, dtype=np.int64)])
    deg = np.bincount(d_all, minlength=n_nodes)
    perm = np.argsort(-deg, kind="stable")  # position -> node, degree desc
    pos = np.empty(n_nodes, np.int64)
    pos[perm] = np.arange(n_nodes)

    nb = -(-n_nodes // P)  # real 128-node blocks
    rounds = -(-nb // n_cores)
    nb_pad = rounds * n_cores
    npos_pad = nb_pad * P
    nbuck = -(-n_nodes // BUCKET_W)

    blocks = np.arange(nb_pad)
    r_of = blocks // n_cores
    j = blocks % n_cores
    core_of = np.where(r_of % 2 == 0, j, n_cores - 1 - j)

    # per-edge: bucket by src window, rank within (dst-position, bucket)
    b_all = s_all // BUCKET_W
    epos = pos[d_all]
    okey = epos * nbuck + b_all
    order = np.argsort(okey, kind="stable")
    ok = okey[order]
    cntpb = np.bincount(okey, minlength=n_nodes * nbuck)
    startpb = np.zeros(n_nodes * nbuck + 1, np.int64)
    np.cumsum(cntpb, out=startpb[1:])
    k = np.arange(ok.size) - startpb[ok]

    # L per (round, bucket) = max over the round's positions of per-bucket count
    cnt2 = np.zeros((npos_pad, nbuck), np.int64)
    cnt2[:n_nodes] = cntpb.reshape(n_nodes, nbuck)
    lrb = cnt2.reshape(rounds, n_cores * P, nbuck).max(axis=1)  # [rounds, nbuck]
    lrb[:, 0] = np.maximum(lrb[:, 0], 1)  # room for dummy fake edges
    lsum = lrb.sum(axis=1)
    col_off = np.zeros(rounds + 1, np.int64)
    np.cumsum(lsum, out=col_off[1:])
    tot_l = int(col_off[-1])
    boff = np.zeros((rounds, nbuck + 1), np.int64)
    np.cumsum(lrb, axis=1, out=boff[:, 1:])

    e_pos = epos[order]
    e_b = b_all[order]
    rr = (e_pos // P) // n_cores
    col = col_off[rr] + boff[rr, e_b] + k
    slot = e_pos % P
    core_e = core_of[e_pos // P]
    sval16 = (s_all[order] - e_b * BUCKET_W).astype(np.int16)

    g16 = np.zeros((n_cores, P, tot_l), np.int16)
    mask_arr = np.zeros((n_cores, P, tot_l), np.float32)
    g16[core_e, slot, col] = sval16
    mask_arr[core_e, slot, col] = 1.0

    # dummy positions get one fake edge (idx 0 of bucket 0, mask 1) so den > 0
    if npos_pad > n_nodes:
        dpos = np.arange(n_nodes, npos_pad)
        dblk = dpos // P
        mask_arr[core_of[dblk], dpos % P, col_off[dblk // n_cores]] = 1.0

    # int16 index streams in dma_gather wrapped-replicated layout:
    # stream position i = col*128 + p ; wrapped (16-partition) and tiled x8
    gs = np.empty((n_cores, P, tot_l * P // 16), np.int16)
    for c in range(n_cores):
        S = g16[c].T.reshape(-1)              # [tot_l*128], col-major positions
        gs[c] = np.tile(S.reshape(-1, 16).T, (8, 1))

    # per-core node lists in round order (node id or -1 for dummy)
    q = np.arange(npos_pad)
    qblk = q // P
    posgrid = np.full((n_cores, rounds * P), -1, np.int64)
    posgrid[core_of[qblk], (qblk // n_cores) * P + q % P] = np.where(
        q < n_nodes, perm[np.minimum(q, n_nodes - 1)], -1)

    return dict(
        rounds=rounds, lsum=lsum.tolist(), col_off=col_off.tolist(),
        boff=boff.tolist(), nbuck=nbuck, tot_l=tot_l, gidx=gs,
        mask_arr=mask_arr, posgrid=posgrid, shard_rows=rounds * P,
    )


# --------------------------------------------------------------------------
# device program (one SPMD NEFF for all cores)
# --------------------------------------------------------------------------
def build_device_program(n_table_pad, shard_rows, rounds, lsum, col_off, boff,
                         nbuck, tot_l, use_act_lrelu=True, nc=None, io=None):
    import concourse.bass as bass
    import concourse.tile as tile
    from concourse import bacc, mybir
    from concourse.masks import make_identity

    FP32 = mybir.dt.float32
    I32 = mybir.dt.int32
    I16 = mybir.dt.int16
    A = mybir.AluOpType
    AF = mybir.ActivationFunctionType

    own_nc = nc is None
    if own_nc:
        nc = bacc.Bacc("TRN2", target_bir_lowering=False, debug=False)
        io = {
            "xT": nc.dram_tensor("xT", [P, n_table_pad], FP32, kind="ExternalInput").ap(),
            "xpT": nc.dram_tensor("xpT", [P, shard_rows], FP32, kind="ExternalInput").ap(),
            "W_src": nc.dram_tensor("W_src", [FIN, HD], FP32, kind="ExternalInput").ap(),
            "W_dst": nc.dram_tensor("W_dst", [FIN, HD], FP32, kind="ExternalInput").ap(),
            "b_src": nc.dram_tensor("b_src", [HD, 1], FP32, kind="ExternalInput").ap(),
            "b_dst": nc.dram_tensor("b_dst", [HD, 1], FP32, kind="ExternalInput").ap(),
            "attn_rep": nc.dram_tensor("attn_rep", [P, HD], FP32, kind="ExternalInput").ap(),
            "gidx": nc.dram_tensor("gidx", [P, tot_l * P // 16], I16,
                                   kind="ExternalInput").ap(),
            "mask": nc.dram_tensor("mask", [P, tot_l], FP32, kind="ExternalInput").ap(),
            "out": nc.dram_tensor("out", [shard_rows, HD], FP32, kind="ExternalOutput").ap(),
        }
    fs_table = nc.dram_tensor("fs_table", [n_table_pad, HD], FP32, kind="Internal").ap()
    fd_shard = nc.dram_tensor("fd_shard", [shard_rows, HD], FP32, kind="Internal").ap()

    def body(tc):
        with ExitStack() as ctx:
            consts = ctx.enter_context(tc.tile_pool(name="consts", bufs=1))
            wsrc_t = consts.tile([FIN, HD], FP32)
            nc.sync.dma_start(wsrc_t[:], io["W_src"][:, :])
            wdst_t = consts.tile([FIN, HD], FP32)
            nc.sync.dma_start(wdst_t[:], io["W_dst"][:, :])
            bsrc_t = consts.tile([HD, 1], FP32)
            nc.sync.dma_start(bsrc_t[:], io["b_src"][:, :])
            bdst_t = consts.tile([HD, 1], FP32)
            nc.sync.dma_start(bdst_t[:], io["b_dst"][:, :])
            attn_t = consts.tile([P, HD], FP32)
            nc.sync.dma_start(attn_t[:], io["attn_rep"][:, :])
            ident = consts.tile([P, P], FP32)
            make_identity(nc, ident[:])

            # ---------------- phase A: feature tables --------------------
            with ExitStack() as actx:
                apool = actx.enter_context(tc.tile_pool(name="pha", bufs=3))
                apsum = actx.enter_context(tc.tile_pool(name="phamm", bufs=2, space="PSUM"))
                tpsum = actx.enter_context(tc.tile_pool(name="phatr", bufs=4, space="PSUM"))

                def phase_a(src_ap, w_t, b_t, dst_table, nrows):
                    for c0 in range(0, nrows, CH):
                        cw = min(CH, nrows - c0)
                        xt = apool.tile([P, CH], FP32, tag="xt")
                        nc.sync.dma_start(xt[:, :cw], src_ap[:, c0:c0 + cw])
                        ps = apsum.tile([P, CH], FP32, tag="mm")
                        nc.tensor.matmul(ps[:, :cw], lhsT=w_t[:], rhs=xt[:, :cw],
                                         start=True, stop=True)
                        fT = apool.tile([P, CH], FP32, tag="fT")
                        nc.vector.tensor_scalar(out=fT[:, :cw], in0=ps[:, :cw],
                                                scalar1=b_t[:, :], scalar2=None,
                                                op0=A.add)
                        for s0 in range(0, cw, P):
                            sw = min(P, cw - s0)
                            pt = tpsum.tile([P, P], FP32, tag="tr")
                            nc.tensor.transpose(out=pt[:sw, :], in_=fT[:, s0:s0 + sw],
                                                identity=ident[:])
                            row = apool.tile([P, P], FP32, tag="row")
                            nc.scalar.copy(out=row[:sw, :], in_=pt[:sw, :])
                            nc.sync.dma_start(dst_table[c0 + s0:c0 + s0 + sw, :],
                                              row[:sw, :])

                phase_a(io["xT"], wsrc_t, bsrc_t, fs_table, n_table_pad)
                phase_a(io["xpT"], wdst_t, bdst_t, fd_shard, shard_rows)

            # ---------------- phase B: per-block message passing ---------
            bpool = ctx.enter_context(tc.tile_pool(name="phb", bufs=2))
            for r in range(rounds):
                ltot = int(lsum[r])
                off = int(col_off[r])
                fd_t = bpool.tile([P, HD], FP32, tag="fd")
                nc.sync.dma_start(fd_t[:], fd_shard[r * P:(r + 1) * P, :])
                den_acc = bpool.tile([P, H], FP32, tag="denacc")
                agg_acc = bpool.tile([P, HD], FP32, tag="aggacc")

                for ci, c0 in enumerate(range(0, ltot, CAP)):
                    Lc = min(CAP, ltot - c0)
                    LH = Lc * H
                    LHD = Lc * HD

                    mask_t = bpool.tile([P, Lc], FP32, tag="mask")
                    nc.sync.dma_start(mask_t[:], io["mask"][:, off + c0:off + c0 + Lc])

                    # bucketed int16 gathers (ANT dma_gather, 4 parallel queues)
                    fs_g = bpool.tile([P, LHD], FP32, tag="fsg")
                    for b in range(nbuck):
                        b0 = max(c0, int(boff[r][b]))
                        b1 = min(c0 + Lc, int(boff[r][b + 1]))
                        if b1 <= b0:
                            continue
                        # sub-calls of <=GCOLS columns to bound the SWDGE
                        # descriptor-ring footprint per instruction
                        for s0 in range(b0, b1, GCOLS):
                            s1 = min(s0 + GCOLS, b1)
                            npos = (s1 - s0) * P
                            gt = bpool.tile([P, GCOLS * P // 16], I16, tag="gidx")
                            p0 = (off + s0) * P
                            nc.sync.dma_start(gt[:, :npos // 16],
                                              io["gidx"][:, p0 // 16:(p0 + npos) // 16])
                            nc.gpsimd.dma_gather(
                                out_ap=fs_g[:, (s0 - c0) * HD:(s1 - c0) * HD]
                                    .rearrange("p (k d) -> p k d", d=HD),
                                in_ap=fs_table[b * BUCKET_W:, :],
                                idxs_ap=gt[:, :npos // 16],
                                num_idxs=npos, num_idxs_reg=npos,
                                elem_size=HD, queue_num=0)

                    # t = fs_g + broadcast(fd)
                    t = bpool.tile([P, LHD], FP32, tag="t")
                    nc.vector.tensor_tensor(
                        out=t[:].rearrange("p (l f) -> p l f", l=Lc),
                        in0=fs_g[:].rearrange("p (l f) -> p l f", l=Lc),
                        in1=fd_t[:, None, :].to_broadcast([P, Lc, HD]), op=A.add)

                    # u = LeakyReLU(t)
                    u = bpool.tile([P, LHD], FP32, tag="u")
                    if use_act_lrelu:
                        nc.scalar.activation(out=u[:], in_=t[:], func=AF.Lrelu,
                                             alpha=0.2)
                    else:
                        nc.vector.scalar_tensor_tensor(
                            out=u[:], in0=t[:], scalar=0.2, in1=t[:],
                            op0=A.mult, op1=A.max)

                    # v = u * attn ; scr = sum_d v
                    v = bpool.tile([P, LHD], FP32, tag="t")
                    nc.vector.tensor_tensor(
                        out=v[:].rearrange("p (l f) -> p l f", l=Lc),
                        in0=u[:].rearrange("p (l f) -> p l f", l=Lc),
                        in1=attn_t[:, None, :].to_broadcast([P, Lc, HD]), op=A.mult)
                    scr = bpool.tile([P, LH], FP32, tag="scr")
                    nc.vector.tensor_reduce(
                        out=scr[:].rearrange("p (l h) -> p l h", h=H),
                        in_=v[:].rearrange("p (l h d) -> p l h d", h=H, d=D),
                        axis=mybir.AxisListType.X, op=A.add)

                    # es = exp(scr) * mask
                    es0 = bpool.tile([P, LH], FP32, tag="es0")
                    nc.scalar.activation(out=es0[:], in_=scr[:], func=AF.Exp)
                    es = bpool.tile([P, LH], FP32, tag="es")
                    nc.vector.tensor_tensor(
                        out=es[:].rearrange("p (l h) -> p l h", h=H),
                        in0=es0[:].rearrange("p (l h) -> p l h", h=H),
                        in1=mask_t[:, :, None].to_broadcast([P, Lc, H]), op=A.mult)

                    # den_c = sum_l es ; w = es * fs_g ; agg_c = sum_l w
                    den_c = bpool.tile([P, H], FP32, tag="denc")
                    nc.vector.tensor_reduce(
                        out=den_c[:], in_=es[:].rearrange("p (l h) -> p h l", h=H),
                        axis=mybir.AxisListType.X, op=A.add)
                    w = bpool.tile([P, LHD], FP32, tag="u")
                    nc.vector.tensor_tensor(
                        out=w[:].rearrange("p (l h d) -> p l h d", h=H, d=D),
                        in0=fs_g[:].rearrange("p (l h d) -> p l h d", h=H, d=D),
                        in1=es[:].rearrange("p (l h) -> p l h", h=H)[:, :, :, None]
                            .to_broadcast([P, Lc, H, D]),
                        op=A.mult)
                    agg_c = bpool.tile([P, HD], FP32, tag="aggc")
                    nc.vector.tensor_reduce(
                        out=agg_c[:], in_=w[:].rearrange("p (l f) -> p f l", l=Lc),
                        axis=mybir.AxisListType.X, op=A.add)

                    if ci == 0:
                        nc.vector.tensor_copy(out=den_acc[:], in_=den_c[:])
                        nc.vector.tensor_copy(out=agg_acc[:], in_=agg_c[:])
                    else:
                        nc.vector.tensor_add(out=den_acc[:], in0=den_acc[:],
                                             in1=den_c[:])
                        nc.vector.tensor_add(out=agg_acc[:], in0=agg_acc[:],
                                             in1=agg_c[:])

                # out = relu(agg_acc / den_acc)
                rden = bpool.tile([P, H], FP32, tag="rden")
                nc.vector.reciprocal(out=rden[:], in_=den_acc[:])
                sc = bpool.tile([P, HD], FP32, tag="sc")
                nc.vector.tensor_tensor(
                    out=sc[:].rearrange("p (h d) -> p h d", h=H),
                    in0=agg_acc[:].rearrange("p (h d) -> p h d", h=H),
                    in1=rden[:, :, None].to_broadcast([P, H, D]), op=A.mult)
                ot = bpool.tile([P, HD], FP32, tag="ot")
                nc.vector.tensor_scalar(out=ot[:], in0=sc[:], scalar1=0.0,
                                        scalar2=None, op0=A.max)
                nc.sync.dma_start(io["out"][r * P:(r + 1) * P, :], ot[:])

    if own_nc:
        import concourse.tile as tile_mod
        with tile_mod.TileContext(nc) as tc:
            body(tc)
        nc.compile()
        return nc, io
    return body


# --------------------------------------------------------------------------
# full kernel: plan -> build -> run on 8 cores -> assemble
# --------------------------------------------------------------------------
_NC_CACHE = {}


_PLAN_CACHE = {}


def kernel(x, src, dst, W_src, b_src, W_dst, b_dst, attn, _trace=False):
    import hashlib
    n_cores = 8
    n = x.shape[0]
    src = np.asarray(src)
    dst = np.asarray(dst)
    pkey = hashlib.sha1(src.tobytes() + dst.tobytes()).hexdigest()
    plan = _PLAN_CACHE.get(pkey)
    if plan is None:
        plan = build_plan(src, dst, n, n_cores)
        _PLAN_CACHE[pkey] = plan
    rounds, shard_rows, tot_l = plan["rounds"], plan["shard_rows"], plan["tot_l"]
    n_table_pad = -(-n // CH) * CH

    key = (n_table_pad, shard_rows, rounds, tuple(plan["lsum"]),
           tuple(map(tuple, plan["boff"])))
    if key in _NC_CACHE:
        nc, io = _NC_CACHE[key]
    else:
        # use_act_lrelu=False: the ACT engine's Lrelu alpha semantics were
        # measured wrong on HW (9e-2 rel err); the DVE max(0.2t, t) is exact.
        nc, io = build_device_program(
            n_table_pad, shard_rows, rounds, plan["lsum"], plan["col_off"],
            plan["boff"], plan["nbuck"], tot_l,
            use_act_lrelu=os.environ.get("GAT_ACT_LRELU", "0") == "1")
        _NC_CACHE[key] = (nc, io)

    x = np.asarray(x, np.float32)
    xT = np.zeros((P, n_table_pad), np.float32)
    xT[:, :n] = x.T
    attn_rep = np.broadcast_to(np.asarray(attn, np.float32).reshape(1, HD),
                               (P, HD)).copy()
    w_src = np.ascontiguousarray(np.asarray(W_src, np.float32))
    w_dst = np.ascontiguousarray(np.asarray(W_dst, np.float32))
    b_src_c = np.asarray(b_src, np.float32).reshape(HD, 1).copy()
    b_dst_c = np.asarray(b_dst, np.float32).reshape(HD, 1).copy()

    in_maps = []
    for c in range(n_cores):
        nodes = plan["posgrid"][c]
        xp = np.zeros((shard_rows, FIN), np.float32)
        valid = nodes >= 0
        xp[valid] = x[nodes[valid]]
        in_maps.append({
            "xT": xT,
            "xpT": np.ascontiguousarray(xp.T),
            "W_src": w_src, "W_dst": w_dst,
            "b_src": b_src_c, "b_dst": b_dst_c,
            "attn_rep": attn_rep,
            "gidx": np.ascontiguousarray(plan["gidx"][c]),
            "mask": np.ascontiguousarray(plan["mask_arr"][c]),
        })

    from concourse.bass_utils import run_bass_kernel_spmd
    res = run_bass_kernel_spmd(nc, in_maps, core_ids=list(range(n_cores)),
                               trace=_trace, stitch_traces=_trace,
                               trace_cores=list(range(n_cores)) if _trace else None)

    out_full = np.zeros((n, HD), np.float32)
    for c in range(n_cores):
        nodes = plan["posgrid"][c]
        valid = nodes >= 0
        out_full[nodes[valid]] = res.results[c]["out"][valid]
    if _trace:
        return out_full, res
    return out_full
